# revision 9
# baseline (speedup 1.0000x reference)
"""Chamfer loss kernel for Trainium2 (8 NeuronCores via bass/tile).

Problem: gts [4, 8192, 3], preds [4, 8192, 3] (f32)
  P[b,n,m] = ||gts[b,n] - preds[b,m]||
  loss[b] = sum_m min_n P + sum_n min_m P

Strategy:
  - d2[n,m] = |g_n|^2 + |p_m|^2 - 2 g.p  is computed as a K-row matmul on the
    tensor engine by augmenting the coordinates host-side. For speed the f32
    coordinates are split into bf16 hi/lo pairs (v = hi + lo + O(eps^2 v)) and
    all cross products are carried as separate contraction rows (K=16), so the
    PE runs at bf16 rate (1 cycle/row vs 4 for f32) while PSUM accumulates in
    f32; the arithmetic error is ~1e-4 absolute on d2, i.e. ~1e-5 relative on
    the loss.
  - sqrt and the clamp are monotonic, so only the per-row running MIN of d2
    is needed on-device; sqrt + final sums happen host-side on 8k values.
  - Sharding: core (b, h) with b in 0..3, h in 0..1 handles batch b and, for
    both orientations (min over preds per gt / min over gts per pred), the
    4096-row half h of the stationary side against all 8192 moving points.
  - Per core reduce pipeline: PE fills [128, 1024] f32 PSUM chunks (2 matmuls
    of 512); ScalarE copies odd chunks PSUM->SBUF; VectorE tensor_tensor_reduce
    fuses elementwise-min of (even PSUM chunk, odd SBUF chunk) with a running
    row-min accumulator, consuming 2 fresh elements/lane/cycle.
"""

import os
import sys

import numpy as np

if "/opt/trn_rl_repo" not in sys.path:
    sys.path.insert(0, "/opt/trn_rl_repo")

B = 4
N = 8192
HALF = N // 2  # stationary rows per core
PART = 128
NRT = HALF // PART  # 32 row tiles
MM = 512  # moving free dim per matmul
KROWS = 16  # bf16-split contraction rows

VARIANT = os.environ.get("CHAMFER_VARIANT", "v3")
_CACHE = {}

_MIN_OP_NAME = "TTR_MIN_ANT"


def _register_custom_op(name, spec_fn):
    """Register a custom DVE op via the documented `dve_ops.OPS` runtime
    extension point (trainium-docs/custom-instructions/04-custom-dve-api.md).
    The micro-op program is written into the per-NEFF DVE table at compile
    time, so no firmware change is needed. `spec_fn` returns the Spec."""
    import concourse.dve_ops as dve_ops
    from concourse.dve_ops import DveOp, OPS, CUSTOM_DVE_SPECS, _SUB_OPCODE_FOR_NAME
    from concourse.dve_spec import lower, _has_src1
    from concourse.dve_uop import DveOpSpec

    for op in OPS:
        if op.name == name:
            return op
    spec = spec_fn()
    row = dve_ops._CUSTOM_DVE_ROW_BASE + len(OPS)
    assert row < 0x20
    shas = {}
    for ver in ("v3", "v4"):
        try:
            s = DveOpSpec(
                name=name, opcode=row, uops=lower(spec, ver=ver),
                rd1_en=_has_src1(spec),
            )
            shas[ver] = s.sha(ver)
        except Exception:
            pass
    op = DveOp(name, spec, subdim=False, uops_sha=shas)
    OPS.append(op)
    CUSTOM_DVE_SPECS[name] = spec
    _SUB_OPCODE_FOR_NAME[name] = row
    return op


def _register_d2xy_op():
    """out[p,k] = (in0[p,k] - s0[p])^2 + (in1[p,k] - s1[p])^2"""
    from concourse.dve_spec import Spec, Src0, Src1, C0, C1, sq

    def _spec():
        def _ref(in0, in1, c0, c1, c2):
            a = in0.astype(np.float32) - np.asarray(c0, np.float32)
            b = in1.astype(np.float32) - np.asarray(c1, np.float32)
            return a * a + b * b

        return Spec(body=sq(Src0 - C0) + sq(Src1 - C1), reference=_ref)

    return _register_custom_op("D2XY_ANT", _spec)


def _register_d2zmin_op():
    """out[p,k] = (in0[p,k] - s0[p])^2 + in1[p,k]
    accum_out[p] = min(s1[p], min_k out[p,k])"""
    from concourse.dve_spec import Spec, Src0, Src1, C0, C1, sq, AluOp

    def _spec():
        def _ref(in0, in1, c0, c1, c2):
            a = in0.astype(np.float32) - np.asarray(c0, np.float32)
            b = a * a + in1.astype(np.float32)
            acc = np.minimum(
                np.asarray(c1, dtype=np.float32),
                b.reshape(b.shape[0], -1).min(axis=-1, keepdims=True),
            )
            return b, acc

        return Spec(
            body=sq(Src0 - C0) + Src1, accum=AluOp.MIN, accum_init=C1,
            reference=_ref,
        )

    return _register_custom_op("D2ZMIN_ANT", _spec)


def _register_min_op():
    """Register a fused pairwise-min + min-reduce custom DVE op:

      out[p,k]     = min(in0[p,k], in1[p,k])
      accum_out[p] = min(s0[p], min_k out[p,k])

    Uses the documented `dve_ops.OPS` runtime extension point
    (trainium-docs/custom-instructions/04-custom-dve-api.md): the micro-op
    program is written into the per-NEFF DVE table at compile time.
    """
    import concourse.dve_ops as dve_ops
    from concourse.dve_ops import DveOp, OPS, CUSTOM_DVE_SPECS, _SUB_OPCODE_FOR_NAME
    from concourse.dve_spec import Spec, Src0, Src1, C0, minn, AluOp, lower
    from concourse.dve_spec import _has_src1
    from concourse.dve_uop import DveOpSpec

    for op in OPS:
        if op.name == _MIN_OP_NAME:
            return op

    def _ref(in0, in1, c0, c1, c2):
        b = np.minimum(in0.astype(np.float32), in1.astype(np.float32))
        acc = np.minimum(
            np.asarray(c0, dtype=np.float32),
            b.reshape(b.shape[0], -1).min(axis=-1, keepdims=True),
        )
        return b, acc

    spec = Spec(body=minn(Src0, Src1), accum=AluOp.MIN, accum_init=C0, reference=_ref)
    row = dve_ops._CUSTOM_DVE_ROW_BASE + len(OPS)
    assert row < 0x20
    shas = {}
    for ver in ("v3", "v4"):
        try:
            s = DveOpSpec(
                name=_MIN_OP_NAME, opcode=row, uops=lower(spec, ver=ver),
                rd1_en=_has_src1(spec),
            )
            shas[ver] = s.sha(ver)
        except Exception:
            pass
    op = DveOp(_MIN_OP_NAME, spec, subdim=False, uops_sha=shas)
    OPS.append(op)
    CUSTOM_DVE_SPECS[_MIN_OP_NAME] = spec
    _SUB_OPCODE_FOR_NAME[_MIN_OP_NAME] = row
    return op


def _emit_min_reduce(nc, out, in0, in1, seed, accum_out):
    op = _register_min_op()
    return nc.vector._custom_dve(
        op, out=out, in0=in0, in1=in1, s0=seed, accum_out=accum_out
    )


def _build_nc_v1(reps=1):
    """Exact f32 matmul (K=5) + plain DVE tensor_reduce. Slow but exact."""
    import concourse.tile as tile
    from concourse import bacc, mybir

    f32 = mybir.dt.float32
    FCH = 2048
    NCH = N // FCH
    nc = bacc.Bacc("TRN2", target_bir_lowering=False, debug=False)

    W = nc.dram_tensor("W", [2, 5, HALF], f32, kind="ExternalInput").ap()
    X = nc.dram_tensor("X", [2, 5, N], f32, kind="ExternalInput").ap()
    OUT = nc.dram_tensor("OUT", [PART, 2 * NRT], f32, kind="ExternalOutput").ap()

    with tile.TileContext(nc) as tc:
        with (
            tc.tile_pool(name="win", bufs=1) as wpool,
            tc.tile_pool(name="xin", bufs=1) as xpool,
            tc.tile_pool(name="outp", bufs=1) as opool,
            tc.tile_pool(name="cmins", bufs=2) as cpool,
            tc.tile_pool(name="psum", bufs=2, space="PSUM") as ppool,
        ):
            wt, xt = [], []
            for s in range(2):
                w = wpool.tile([5, HALF], f32, tag=f"w{s}")
                x = xpool.tile([5, N], f32, tag=f"x{s}")
                nc.sync.dma_start(w[:], W[s])
                nc.sync.dma_start(x[:], X[s])
                wt.append(w)
                xt.append(x)
            outsb = opool.tile([PART, 2 * NRT], f32)
            for _ in range(reps):
                for s in range(2):
                    for rt in range(NRT):
                        cm = cpool.tile([PART, NCH], f32, tag="cm")
                        for c in range(NCH):
                            pt = ppool.tile([PART, FCH], f32, tag="ps")
                            for j in range(FCH // MM):
                                col = c * FCH + j * MM
                                nc.tensor.matmul(
                                    pt[:, j * MM : (j + 1) * MM],
                                    lhsT=wt[s][:, rt * PART : (rt + 1) * PART],
                                    rhs=xt[s][:, col : col + MM],
                                    start=True,
                                    stop=True,
                                )
                            nc.vector.tensor_reduce(
                                out=cm[:, c : c + 1],
                                in_=pt[:],
                                axis=mybir.AxisListType.X,
                                op=mybir.AluOpType.min,
                            )
                        oc = s * NRT + rt
                        nc.vector.tensor_reduce(
                            out=outsb[:, oc : oc + 1],
                            in_=cm[:],
                            axis=mybir.AxisListType.X,
                            op=mybir.AluOpType.min,
                        )
            nc.sync.dma_start(OUT[:, :], outsb[:])
    nc.compile()
    return nc


def _build_nc_v2(reps=1):
    """bf16-split K=16 matmul + ScalarE copy + fused TTR min pipeline."""
    import concourse.tile as tile
    from concourse import bacc, mybir

    f32 = mybir.dt.float32
    bf16 = mybir.dt.bfloat16
    FCH = 1024  # psum chunk (2 banks f32)
    NCH = N // FCH  # 8 chunks -> 4 pairs per row tile
    NPAIR = NCH // 2
    BIG = 3.0e38

    nc = bacc.Bacc("TRN2", target_bir_lowering=False, debug=False)

    W = nc.dram_tensor("W", [2, KROWS, HALF], bf16, kind="ExternalInput").ap()
    X = nc.dram_tensor("X", [2, KROWS, N], bf16, kind="ExternalInput").ap()
    OUT = nc.dram_tensor("OUT", [PART, 2 * NRT], f32, kind="ExternalOutput").ap()

    with tile.TileContext(nc) as tc:
        with (
            tc.tile_pool(name="win", bufs=1) as wpool,
            tc.tile_pool(name="xin", bufs=1) as xpool,
            tc.tile_pool(name="outp", bufs=1) as opool,
            tc.tile_pool(name="sodd", bufs=3) as spool,
            tc.tile_pool(name="scr", bufs=2) as rpool,
            tc.tile_pool(name="accp", bufs=2) as apool,
            tc.tile_pool(name="psum", bufs=4, space="PSUM") as ppool,
        ):
            wt, xt = [], []
            for s in range(2):
                w = wpool.tile([KROWS, HALF], bf16, tag=f"w{s}")
                x = xpool.tile([KROWS, N], bf16, tag=f"x{s}")
                nc.sync.dma_start(w[:], W[s])
                nc.sync.dma_start(x[:], X[s])
                wt.append(w)
                xt.append(x)
            outsb = opool.tile([PART, 2 * NRT], f32)
            for _ in range(reps):
                for s in range(2):
                    for rt in range(NRT):
                        lhsT = wt[s][:, rt * PART : (rt + 1) * PART]
                        acc = apool.tile([PART, NPAIR - 1], f32, tag="acc")
                        for pr in range(NPAIR):
                            pts = []
                            for half in range(2):
                                c = 2 * pr + half
                                pt = ppool.tile([PART, FCH], f32, tag="ps")
                                for j in range(FCH // MM):
                                    col = c * FCH + j * MM
                                    nc.tensor.matmul(
                                        pt[:, j * MM : (j + 1) * MM],
                                        lhsT=lhsT,
                                        rhs=xt[s][:, col : col + MM],
                                        start=True,
                                        stop=True,
                                    )
                                pts.append(pt)
                            sb = spool.tile([PART, FCH], f32, tag="sodd")
                            nc.scalar.copy(sb[:], pts[1][:])
                            scr = rpool.tile([PART, FCH], f32, tag="scr")
                            init = BIG if pr == 0 else acc[:, pr - 1 : pr]
                            if pr < NPAIR - 1:
                                aout = acc[:, pr : pr + 1]
                            else:
                                oc = s * NRT + rt
                                aout = outsb[:, oc : oc + 1]
                            _emit_min_reduce(
                                nc,
                                out=scr[:],
                                in0=pts[0][:],
                                in1=sb[:],
                                seed=init,
                                accum_out=aout,
                            )
            nc.sync.dma_start(OUT[:, :], outsb[:])
    nc.compile()
    return nc


def _build_nc_v3(reps=1):
    """Pure-DVE design: per 128-row stationary tile, two fused custom DVE ops
    compute d2 = (px-gx)^2 + (py-gy)^2 + (pz-gz)^2 over all 8192 moving
    points (held broadcast across partitions) and fold the row-min in the
    same instruction. ~137 instructions per core; no PE/PSUM use at all."""
    import concourse.tile as tile
    from concourse import bacc, mybir

    f32 = mybir.dt.float32
    BIG = 3.0e38
    op_xy = _register_d2xy_op()
    op_zmin = _register_d2zmin_op()

    nc = bacc.Bacc("TRN2", target_bir_lowering=False, debug=False)
    # moving coords, one row per (orientation, coord)
    X = nc.dram_tensor("X", [2, 3, N], f32, kind="ExternalInput").ap()
    # stationary coords: [128, (s*3+d)*NRT + rt]
    G = nc.dram_tensor("G", [PART, 6 * NRT], f32, kind="ExternalInput").ap()
    OUT = nc.dram_tensor("OUT", [PART, 2 * NRT], f32, kind="ExternalOutput").ap()

    with tile.TileContext(nc) as tc:
        with (
            tc.tile_pool(name="stat", bufs=1) as gpool,
            tc.tile_pool(name="bcast", bufs=1) as bpool,
            tc.tile_pool(name="tmid", bufs=1) as tpool,
            tc.tile_pool(name="scr", bufs=1) as rpool,
            tc.tile_pool(name="outp", bufs=1) as opool,
        ):
            gst = gpool.tile([PART, 6 * NRT], f32, tag="g")
            nc.sync.dma_start(gst[:], G[:, :])
            outsb = opool.tile([PART, 2 * NRT], f32, tag="o")
            for _ in range(reps):
                for s in range(2):
                    bc = []
                    for d in range(3):
                        b = bpool.tile([PART, N], f32, tag=f"b{d}")
                        nc.sync.dma_start(
                            b[:], X[s, d : d + 1, :].broadcast_to((PART, N))
                        )
                        bc.append(b)
                    for rt in range(NRT):
                        gx = gst[:, (s * 3 + 0) * NRT + rt : (s * 3 + 0) * NRT + rt + 1]
                        gy = gst[:, (s * 3 + 1) * NRT + rt : (s * 3 + 1) * NRT + rt + 1]
                        gz = gst[:, (s * 3 + 2) * NRT + rt : (s * 3 + 2) * NRT + rt + 1]
                        t = tpool.tile([PART, N], f32, tag="t")
                        nc.vector._custom_dve(
                            op_xy, out=t[:], in0=bc[0][:], in1=bc[1][:],
                            s0=gx, s1=gy,
                        )
                        scr = rpool.tile([PART, N], f32, tag="scr")
                        oc = s * NRT + rt
                        nc.vector._custom_dve(
                            op_zmin, out=scr[:], in0=bc[2][:], in1=t[:],
                            s0=gz, s1=BIG,
                            accum_out=outsb[:, oc : oc + 1],
                        )
            nc.sync.dma_start(OUT[:, :], outsb[:])
    nc.compile()
    return nc


def get_nc(variant=None, reps=1):
    variant = variant or VARIANT
    key = (variant, reps)
    if key not in _CACHE:
        _CACHE[key] = {
            "v1": _build_nc_v1,
            "v2": _build_nc_v2,
            "v3": _build_nc_v3,
        }[variant](reps)
    return _CACHE[key]


def _split_bf16(v32):
    import ml_dtypes

    bf = ml_dtypes.bfloat16
    hi = v32.astype(bf)
    lo = (v32 - hi.astype(np.float32)).astype(bf)
    return hi, lo


def _make_in_maps_v1(gts, preds):
    ones_n = np.ones(N, dtype=np.float32)
    ones_h = np.ones(HALF, dtype=np.float32)
    in_maps = []
    for b in range(B):
        g = gts[b]
        p = preds[b]
        gg = (g * g).sum(-1).astype(np.float32)
        pp = (p * p).sum(-1).astype(np.float32)
        X0 = np.stack([-2 * p[:, 0], -2 * p[:, 1], -2 * p[:, 2], ones_n, pp])
        X1 = np.stack([-2 * g[:, 0], -2 * g[:, 1], -2 * g[:, 2], ones_n, gg])
        X = np.stack([X0, X1]).astype(np.float32)
        for h in range(2):
            sl = slice(h * HALF, (h + 1) * HALF)
            W0 = np.stack([g[sl, 0], g[sl, 1], g[sl, 2], gg[sl], ones_h])
            W1 = np.stack([p[sl, 0], p[sl, 1], p[sl, 2], pp[sl], ones_h])
            Wm = np.stack([W0, W1]).astype(np.float32)
            in_maps.append(
                {"W": np.ascontiguousarray(Wm), "X": np.ascontiguousarray(X)}
            )
    return in_maps


def _aug_split(c, cc, moving):
    """Build the 16 bf16 rows for one point set.

    c: [M, 3] f32 coords, cc: [M] f32 squared norms.
    moving=False -> stationary rows:  ghi(x,y,z) x3, glo(x,y,z) x3 paired as
      below, cc_hi, cc_lo, 1, 1
    moving=True  -> moving rows for the OTHER side's stationary:
      -2*phi, -2*plo etc, 1, 1, pp_hi, pp_lo
    Row pairing (W row k) . (X row k):
      0-2:  g_hi . -2 p_hi
      3-5:  g_hi . -2 p_lo
      6-8:  g_lo . -2 p_hi
      9-11: g_lo . -2 p_lo
      12:   gg_hi . 1
      13:   gg_lo . 1
      14:   1 . pp_hi
      15:   1 . pp_lo
    """
    import ml_dtypes

    bf = ml_dtypes.bfloat16
    M = c.shape[0]
    hi, lo = _split_bf16(c)  # [M,3] each
    cc_hi, cc_lo = _split_bf16(cc)
    one = np.ones(M, dtype=bf)
    rows = np.empty((KROWS, M), dtype=bf)
    if not moving:
        for d in range(3):
            rows[d] = hi[:, d]
            rows[3 + d] = hi[:, d]
            rows[6 + d] = lo[:, d]
            rows[9 + d] = lo[:, d]
        rows[12] = cc_hi
        rows[13] = cc_lo
        rows[14] = one
        rows[15] = one
    else:
        m2hi = (-2.0 * hi.astype(np.float32)).astype(bf)
        m2lo = (-2.0 * lo.astype(np.float32)).astype(bf)
        for d in range(3):
            rows[d] = m2hi[:, d]
            rows[3 + d] = m2lo[:, d]
            rows[6 + d] = m2hi[:, d]
            rows[9 + d] = m2lo[:, d]
        rows[12] = one
        rows[13] = one
        rows[14] = cc_hi
        rows[15] = cc_lo
    return rows


def _make_in_maps_v2(gts, preds):
    in_maps = []
    for b in range(B):
        g = gts[b]
        p = preds[b]
        gg = (g * g).sum(-1).astype(np.float32)
        pp = (p * p).sum(-1).astype(np.float32)
        X0 = _aug_split(p, pp, moving=True)  # vs stationary gts
        X1 = _aug_split(g, gg, moving=True)  # vs stationary preds
        X = np.stack([X0, X1])
        for h in range(2):
            sl = slice(h * HALF, (h + 1) * HALF)
            W0 = _aug_split(g[sl], gg[sl], moving=False)
            W1 = _aug_split(p[sl], pp[sl], moving=False)
            Wm = np.stack([W0, W1])
            in_maps.append(
                {"W": np.ascontiguousarray(Wm), "X": np.ascontiguousarray(X)}
            )
    return in_maps


def _make_in_maps_v3(gts, preds):
    in_maps = []
    for b in range(B):
        g = gts[b]  # [N, 3]
        p = preds[b]
        # moving side: orientation 0 scans preds, orientation 1 scans gts
        X = np.stack([p.T, g.T]).astype(np.float32)  # [2, 3, N]
        for h in range(2):
            sl = slice(h * HALF, (h + 1) * HALF)
            # stationary: [128, (s*3+d)*NRT + rt] = coord d of point rt*128+row
            gh = g[sl].reshape(NRT, PART, 3)  # [rt, p, d]
            ph = p[sl].reshape(NRT, PART, 3)
            Gm = np.empty((PART, 6 * NRT), dtype=np.float32)
            for d in range(3):
                Gm[:, (0 * 3 + d) * NRT : (0 * 3 + d) * NRT + NRT] = gh[:, :, d].T
                Gm[:, (1 * 3 + d) * NRT : (1 * 3 + d) * NRT + NRT] = ph[:, :, d].T
            in_maps.append(
                {"X": np.ascontiguousarray(X), "G": np.ascontiguousarray(Gm)}
            )
    return in_maps


def _make_in_maps(gts, preds, variant=None):
    variant = variant or VARIANT
    gts = np.asarray(gts, dtype=np.float32)
    preds = np.asarray(preds, dtype=np.float32)
    fn = {"v1": _make_in_maps_v1, "v2": _make_in_maps_v2, "v3": _make_in_maps_v3}[
        variant
    ]
    return fn(gts, preds)


def _combine(results):
    # OUT [128, 64]: col s*NRT + rt holds rowmins of stationary rows
    # rt*128 .. rt*128+127 for orientation s, on the core's half.
    loss = np.zeros(B, dtype=np.float32)
    for b in range(B):
        per_s = []
        for s in range(2):
            halves = []
            for h in range(2):
                out = results[2 * b + h]["OUT"]  # [128, 64]
                sub = out[:, s * NRT : (s + 1) * NRT]  # [128, 32]
                halves.append(sub.T.reshape(-1))  # [4096] rowmin d2
            per_s.append(np.concatenate(halves))  # [8192]
        d2 = np.concatenate(per_s)
        d = np.sqrt(np.maximum(d2.astype(np.float64), 0.0))
        loss[b] = np.float32(d.sum())
    return loss


def _run(in_maps, variant=None, reps=1, trace=False):
    from concourse.bass_utils import run_bass_kernel_spmd

    nc = get_nc(variant, reps)
    return run_bass_kernel_spmd(nc, in_maps, core_ids=list(range(8)), trace=trace)


def kernel(gts, preds):
    in_maps = _make_in_maps(gts, preds)
    res = _run(in_maps)
    return _combine(res.results)


# revision 11
# speedup vs baseline: 7.1637x; 7.1637x over previous
"""Chamfer loss kernel for Trainium2 (8 NeuronCores via bass/tile).

Problem: gts [4, 8192, 3], preds [4, 8192, 3] (f32)
  P[b,n,m] = ||gts[b,n] - preds[b,m]||
  loss[b] = sum_m min_n P + sum_n min_m P

Strategy:
  - d2[n,m] = |g_n|^2 + |p_m|^2 - 2 g.p  is computed as a K-row matmul on the
    tensor engine by augmenting the coordinates host-side. For speed the f32
    coordinates are split into bf16 hi/lo pairs (v = hi + lo + O(eps^2 v)) and
    all cross products are carried as separate contraction rows (K=16), so the
    PE runs at bf16 rate (1 cycle/row vs 4 for f32) while PSUM accumulates in
    f32; the arithmetic error is ~1e-4 absolute on d2, i.e. ~1e-5 relative on
    the loss.
  - sqrt and the clamp are monotonic, so only the per-row running MIN of d2
    is needed on-device; sqrt + final sums happen host-side on 8k values.
  - Sharding: core (b, h) with b in 0..3, h in 0..1 handles batch b and, for
    both orientations (min over preds per gt / min over gts per pred), the
    4096-row half h of the stationary side against all 8192 moving points.
  - Per core reduce pipeline: PE fills [128, 1024] f32 PSUM chunks (2 matmuls
    of 512); ScalarE copies odd chunks PSUM->SBUF; VectorE tensor_tensor_reduce
    fuses elementwise-min of (even PSUM chunk, odd SBUF chunk) with a running
    row-min accumulator, consuming 2 fresh elements/lane/cycle.
"""

import os
import sys

import numpy as np

if "/opt/trn_rl_repo" not in sys.path:
    sys.path.insert(0, "/opt/trn_rl_repo")

B = 4
N = 8192
HALF = N // 2  # stationary rows per core
PART = 128
NRT = HALF // PART  # 32 row tiles
MM = 512  # moving free dim per matmul
KROWS = 16  # bf16-split contraction rows

VARIANT = os.environ.get("CHAMFER_VARIANT", "v3")
_CACHE = {}

_MIN_OP_NAME = "TTR_MIN_ANT"


def _register_custom_op(name, spec_fn):
    """Register a custom DVE op via the documented `dve_ops.OPS` runtime
    extension point (trainium-docs/custom-instructions/04-custom-dve-api.md).
    The micro-op program is written into the per-NEFF DVE table at compile
    time, so no firmware change is needed. `spec_fn` returns the Spec."""
    import concourse.dve_ops as dve_ops
    from concourse.dve_ops import DveOp, OPS, CUSTOM_DVE_SPECS, _SUB_OPCODE_FOR_NAME
    from concourse.dve_spec import lower, _has_src1
    from concourse.dve_uop import DveOpSpec

    for op in OPS:
        if op.name == name:
            return op
    spec = spec_fn()
    row = dve_ops._CUSTOM_DVE_ROW_BASE + len(OPS)
    assert row < 0x20
    shas = {}
    for ver in ("v3", "v4"):
        try:
            s = DveOpSpec(
                name=name, opcode=row, uops=lower(spec, ver=ver),
                rd1_en=_has_src1(spec),
            )
            shas[ver] = s.sha(ver)
        except Exception:
            pass
    op = DveOp(name, spec, subdim=False, uops_sha=shas)
    OPS.append(op)
    CUSTOM_DVE_SPECS[name] = spec
    _SUB_OPCODE_FOR_NAME[name] = row
    return op


def _register_d2xy_op():
    """out[p,k] = (in0[p,k] - s0[p])^2 + (in1[p,k] - s1[p])^2"""
    from concourse.dve_spec import Spec, Src0, Src1, C0, C1, sq

    def _spec():
        def _ref(in0, in1, c0, c1, c2):
            a = in0.astype(np.float32) - np.asarray(c0, np.float32)
            b = in1.astype(np.float32) - np.asarray(c1, np.float32)
            return a * a + b * b

        return Spec(body=sq(Src0 - C0) + sq(Src1 - C1), reference=_ref)

    return _register_custom_op("D2XY_ANT", _spec)


def _register_d2zmin_op():
    """out[p,k] = (in0[p,k] - s0[p])^2 + in1[p,k]
    accum_out[p] = min(s1[p], min_k out[p,k])"""
    from concourse.dve_spec import Spec, Src0, Src1, C0, C1, sq, AluOp

    def _spec():
        def _ref(in0, in1, c0, c1, c2):
            a = in0.astype(np.float32) - np.asarray(c0, np.float32)
            b = a * a + in1.astype(np.float32)
            acc = np.minimum(
                np.asarray(c1, dtype=np.float32),
                b.reshape(b.shape[0], -1).min(axis=-1, keepdims=True),
            )
            return b, acc

        return Spec(
            body=sq(Src0 - C0) + Src1, accum=AluOp.MIN, accum_init=C1,
            reference=_ref,
        )

    return _register_custom_op("D2ZMIN_ANT", _spec)


def _register_min_op():
    """Register a fused pairwise-min + min-reduce custom DVE op:

      out[p,k]     = min(in0[p,k], in1[p,k])
      accum_out[p] = min(s0[p], min_k out[p,k])

    Uses the documented `dve_ops.OPS` runtime extension point
    (trainium-docs/custom-instructions/04-custom-dve-api.md): the micro-op
    program is written into the per-NEFF DVE table at compile time.
    """
    import concourse.dve_ops as dve_ops
    from concourse.dve_ops import DveOp, OPS, CUSTOM_DVE_SPECS, _SUB_OPCODE_FOR_NAME
    from concourse.dve_spec import Spec, Src0, Src1, C0, minn, AluOp, lower
    from concourse.dve_spec import _has_src1
    from concourse.dve_uop import DveOpSpec

    for op in OPS:
        if op.name == _MIN_OP_NAME:
            return op

    def _ref(in0, in1, c0, c1, c2):
        b = np.minimum(in0.astype(np.float32), in1.astype(np.float32))
        acc = np.minimum(
            np.asarray(c0, dtype=np.float32),
            b.reshape(b.shape[0], -1).min(axis=-1, keepdims=True),
        )
        return b, acc

    spec = Spec(body=minn(Src0, Src1), accum=AluOp.MIN, accum_init=C0, reference=_ref)
    row = dve_ops._CUSTOM_DVE_ROW_BASE + len(OPS)
    assert row < 0x20
    shas = {}
    for ver in ("v3", "v4"):
        try:
            s = DveOpSpec(
                name=_MIN_OP_NAME, opcode=row, uops=lower(spec, ver=ver),
                rd1_en=_has_src1(spec),
            )
            shas[ver] = s.sha(ver)
        except Exception:
            pass
    op = DveOp(_MIN_OP_NAME, spec, subdim=False, uops_sha=shas)
    OPS.append(op)
    CUSTOM_DVE_SPECS[_MIN_OP_NAME] = spec
    _SUB_OPCODE_FOR_NAME[_MIN_OP_NAME] = row
    return op


def _emit_min_reduce(nc, out, in0, in1, seed, accum_out):
    op = _register_min_op()
    return nc.vector._custom_dve(
        op, out=out, in0=in0, in1=in1, s0=seed, accum_out=accum_out
    )


def _build_nc_v1(reps=1):
    """Exact f32 matmul (K=5) + plain DVE tensor_reduce. Slow but exact."""
    import concourse.tile as tile
    from concourse import bacc, mybir

    f32 = mybir.dt.float32
    FCH = 2048
    NCH = N // FCH
    nc = bacc.Bacc("TRN2", target_bir_lowering=False, debug=False)

    W = nc.dram_tensor("W", [2, 5, HALF], f32, kind="ExternalInput").ap()
    X = nc.dram_tensor("X", [2, 5, N], f32, kind="ExternalInput").ap()
    OUT = nc.dram_tensor("OUT", [PART, 2 * NRT], f32, kind="ExternalOutput").ap()

    with tile.TileContext(nc) as tc:
        with (
            tc.tile_pool(name="win", bufs=1) as wpool,
            tc.tile_pool(name="xin", bufs=1) as xpool,
            tc.tile_pool(name="outp", bufs=1) as opool,
            tc.tile_pool(name="cmins", bufs=2) as cpool,
            tc.tile_pool(name="psum", bufs=2, space="PSUM") as ppool,
        ):
            wt, xt = [], []
            for s in range(2):
                w = wpool.tile([5, HALF], f32, tag=f"w{s}")
                x = xpool.tile([5, N], f32, tag=f"x{s}")
                nc.sync.dma_start(w[:], W[s])
                nc.sync.dma_start(x[:], X[s])
                wt.append(w)
                xt.append(x)
            outsb = opool.tile([PART, 2 * NRT], f32)
            for _ in range(reps):
                for s in range(2):
                    for rt in range(NRT):
                        cm = cpool.tile([PART, NCH], f32, tag="cm")
                        for c in range(NCH):
                            pt = ppool.tile([PART, FCH], f32, tag="ps")
                            for j in range(FCH // MM):
                                col = c * FCH + j * MM
                                nc.tensor.matmul(
                                    pt[:, j * MM : (j + 1) * MM],
                                    lhsT=wt[s][:, rt * PART : (rt + 1) * PART],
                                    rhs=xt[s][:, col : col + MM],
                                    start=True,
                                    stop=True,
                                )
                            nc.vector.tensor_reduce(
                                out=cm[:, c : c + 1],
                                in_=pt[:],
                                axis=mybir.AxisListType.X,
                                op=mybir.AluOpType.min,
                            )
                        oc = s * NRT + rt
                        nc.vector.tensor_reduce(
                            out=outsb[:, oc : oc + 1],
                            in_=cm[:],
                            axis=mybir.AxisListType.X,
                            op=mybir.AluOpType.min,
                        )
            nc.sync.dma_start(OUT[:, :], outsb[:])
    nc.compile()
    return nc


def _build_nc_v2(reps=1):
    """bf16-split K=16 matmul + ScalarE copy + fused TTR min pipeline."""
    import concourse.tile as tile
    from concourse import bacc, mybir

    f32 = mybir.dt.float32
    bf16 = mybir.dt.bfloat16
    FCH = 1024  # psum chunk (2 banks f32)
    NCH = N // FCH  # 8 chunks -> 4 pairs per row tile
    NPAIR = NCH // 2
    BIG = 3.0e38

    nc = bacc.Bacc("TRN2", target_bir_lowering=False, debug=False)

    W = nc.dram_tensor("W", [2, KROWS, HALF], bf16, kind="ExternalInput").ap()
    X = nc.dram_tensor("X", [2, KROWS, N], bf16, kind="ExternalInput").ap()
    OUT = nc.dram_tensor("OUT", [PART, 2 * NRT], f32, kind="ExternalOutput").ap()

    with tile.TileContext(nc) as tc:
        with (
            tc.tile_pool(name="win", bufs=1) as wpool,
            tc.tile_pool(name="xin", bufs=1) as xpool,
            tc.tile_pool(name="outp", bufs=1) as opool,
            tc.tile_pool(name="sodd", bufs=3) as spool,
            tc.tile_pool(name="scr", bufs=2) as rpool,
            tc.tile_pool(name="accp", bufs=2) as apool,
            tc.tile_pool(name="psum", bufs=4, space="PSUM") as ppool,
        ):
            wt, xt = [], []
            for s in range(2):
                w = wpool.tile([KROWS, HALF], bf16, tag=f"w{s}")
                x = xpool.tile([KROWS, N], bf16, tag=f"x{s}")
                nc.sync.dma_start(w[:], W[s])
                nc.sync.dma_start(x[:], X[s])
                wt.append(w)
                xt.append(x)
            outsb = opool.tile([PART, 2 * NRT], f32)
            for _ in range(reps):
                for s in range(2):
                    for rt in range(NRT):
                        lhsT = wt[s][:, rt * PART : (rt + 1) * PART]
                        acc = apool.tile([PART, NPAIR - 1], f32, tag="acc")
                        for pr in range(NPAIR):
                            pts = []
                            for half in range(2):
                                c = 2 * pr + half
                                pt = ppool.tile([PART, FCH], f32, tag="ps")
                                for j in range(FCH // MM):
                                    col = c * FCH + j * MM
                                    nc.tensor.matmul(
                                        pt[:, j * MM : (j + 1) * MM],
                                        lhsT=lhsT,
                                        rhs=xt[s][:, col : col + MM],
                                        start=True,
                                        stop=True,
                                    )
                                pts.append(pt)
                            sb = spool.tile([PART, FCH], f32, tag="sodd")
                            nc.scalar.copy(sb[:], pts[1][:])
                            scr = rpool.tile([PART, FCH], f32, tag="scr")
                            init = BIG if pr == 0 else acc[:, pr - 1 : pr]
                            if pr < NPAIR - 1:
                                aout = acc[:, pr : pr + 1]
                            else:
                                oc = s * NRT + rt
                                aout = outsb[:, oc : oc + 1]
                            _emit_min_reduce(
                                nc,
                                out=scr[:],
                                in0=pts[0][:],
                                in1=sb[:],
                                seed=init,
                                accum_out=aout,
                            )
            nc.sync.dma_start(OUT[:, :], outsb[:])
    nc.compile()
    return nc


def _build_nc_v3(reps=1):
    """Pure-DVE design: per 128-row stationary tile, two fused custom DVE ops
    compute d2 = (px-gx)^2 + (py-gy)^2 + (pz-gz)^2 over all 8192 moving
    points (held broadcast across partitions) and fold the row-min in the
    same instruction. ~137 instructions per core; no PE/PSUM use at all."""
    import concourse.tile as tile
    from concourse import bacc, mybir

    f32 = mybir.dt.float32
    BIG = 3.0e38
    op_xy = _register_d2xy_op()
    op_zmin = _register_d2zmin_op()

    nc = bacc.Bacc("TRN2", target_bir_lowering=False, debug=False)
    # moving coords, one row per (orientation, coord)
    X = nc.dram_tensor("X", [2, 3, N], f32, kind="ExternalInput").ap()
    # stationary coords: [128, (s*3+d)*NRT + rt]
    G = nc.dram_tensor("G", [PART, 6 * NRT], f32, kind="ExternalInput").ap()
    OUT = nc.dram_tensor("OUT", [PART, 2 * NRT], f32, kind="ExternalOutput").ap()

    with tile.TileContext(nc) as tc:
        with (
            tc.tile_pool(name="stat", bufs=1) as gpool,
            tc.tile_pool(name="bcast", bufs=1) as bpool,
            tc.tile_pool(name="tmid", bufs=1) as tpool,
            tc.tile_pool(name="scr", bufs=1) as rpool,
            tc.tile_pool(name="outp", bufs=1) as opool,
        ):
            gst = gpool.tile([PART, 6 * NRT], f32, tag="g")
            nc.sync.dma_start(gst[:], G[:, :])
            outsb = opool.tile([PART, 2 * NRT], f32, tag="o")
            for _ in range(reps):
                for s in range(2):
                    bc = []
                    for d in range(3):
                        b = bpool.tile([PART, N], f32, tag=f"b{d}")
                        nc.sync.dma_start(
                            b[:], X[s, d : d + 1, :].broadcast_to((PART, N))
                        )
                        bc.append(b)
                    for rt in range(NRT):
                        gx = gst[:, (s * 3 + 0) * NRT + rt : (s * 3 + 0) * NRT + rt + 1]
                        gy = gst[:, (s * 3 + 1) * NRT + rt : (s * 3 + 1) * NRT + rt + 1]
                        gz = gst[:, (s * 3 + 2) * NRT + rt : (s * 3 + 2) * NRT + rt + 1]
                        t = tpool.tile([PART, N], f32, tag="t")
                        nc.vector._custom_dve(
                            op_xy, out=t[:], in0=bc[0][:], in1=bc[1][:],
                            s0=gx, s1=gy,
                        )
                        scr = rpool.tile([PART, N], f32, tag="scr")
                        oc = s * NRT + rt
                        nc.vector._custom_dve(
                            op_zmin, out=scr[:], in0=bc[2][:], in1=t[:],
                            s0=gz, s1=BIG,
                            accum_out=outsb[:, oc : oc + 1],
                        )
            nc.sync.dma_start(OUT[:, :], outsb[:])
    nc.compile()
    return nc


def get_nc(variant=None, reps=1):
    variant = variant or VARIANT
    key = (variant, reps)
    if key not in _CACHE:
        _CACHE[key] = {
            "v1": _build_nc_v1,
            "v2": _build_nc_v2,
            "v3": _build_nc_v3,
        }[variant](reps)
    return _CACHE[key]


def _split_bf16(v32):
    import ml_dtypes

    bf = ml_dtypes.bfloat16
    hi = v32.astype(bf)
    lo = (v32 - hi.astype(np.float32)).astype(bf)
    return hi, lo


def _make_in_maps_v1(gts, preds):
    ones_n = np.ones(N, dtype=np.float32)
    ones_h = np.ones(HALF, dtype=np.float32)
    in_maps = []
    for b in range(B):
        g = gts[b]
        p = preds[b]
        gg = (g * g).sum(-1).astype(np.float32)
        pp = (p * p).sum(-1).astype(np.float32)
        X0 = np.stack([-2 * p[:, 0], -2 * p[:, 1], -2 * p[:, 2], ones_n, pp])
        X1 = np.stack([-2 * g[:, 0], -2 * g[:, 1], -2 * g[:, 2], ones_n, gg])
        X = np.stack([X0, X1]).astype(np.float32)
        for h in range(2):
            sl = slice(h * HALF, (h + 1) * HALF)
            W0 = np.stack([g[sl, 0], g[sl, 1], g[sl, 2], gg[sl], ones_h])
            W1 = np.stack([p[sl, 0], p[sl, 1], p[sl, 2], pp[sl], ones_h])
            Wm = np.stack([W0, W1]).astype(np.float32)
            in_maps.append(
                {"W": np.ascontiguousarray(Wm), "X": np.ascontiguousarray(X)}
            )
    return in_maps


def _aug_split(c, cc, moving):
    """Build the 16 bf16 rows for one point set.

    c: [M, 3] f32 coords, cc: [M] f32 squared norms.
    moving=False -> stationary rows:  ghi(x,y,z) x3, glo(x,y,z) x3 paired as
      below, cc_hi, cc_lo, 1, 1
    moving=True  -> moving rows for the OTHER side's stationary:
      -2*phi, -2*plo etc, 1, 1, pp_hi, pp_lo
    Row pairing (W row k) . (X row k):
      0-2:  g_hi . -2 p_hi
      3-5:  g_hi . -2 p_lo
      6-8:  g_lo . -2 p_hi
      9-11: g_lo . -2 p_lo
      12:   gg_hi . 1
      13:   gg_lo . 1
      14:   1 . pp_hi
      15:   1 . pp_lo
    """
    import ml_dtypes

    bf = ml_dtypes.bfloat16
    M = c.shape[0]
    hi, lo = _split_bf16(c)  # [M,3] each
    cc_hi, cc_lo = _split_bf16(cc)
    one = np.ones(M, dtype=bf)
    rows = np.empty((KROWS, M), dtype=bf)
    if not moving:
        for d in range(3):
            rows[d] = hi[:, d]
            rows[3 + d] = hi[:, d]
            rows[6 + d] = lo[:, d]
            rows[9 + d] = lo[:, d]
        rows[12] = cc_hi
        rows[13] = cc_lo
        rows[14] = one
        rows[15] = one
    else:
        m2hi = (-2.0 * hi.astype(np.float32)).astype(bf)
        m2lo = (-2.0 * lo.astype(np.float32)).astype(bf)
        for d in range(3):
            rows[d] = m2hi[:, d]
            rows[3 + d] = m2lo[:, d]
            rows[6 + d] = m2hi[:, d]
            rows[9 + d] = m2lo[:, d]
        rows[12] = one
        rows[13] = one
        rows[14] = cc_hi
        rows[15] = cc_lo
    return rows


def _make_in_maps_v2(gts, preds):
    in_maps = []
    for b in range(B):
        g = gts[b]
        p = preds[b]
        gg = (g * g).sum(-1).astype(np.float32)
        pp = (p * p).sum(-1).astype(np.float32)
        X0 = _aug_split(p, pp, moving=True)  # vs stationary gts
        X1 = _aug_split(g, gg, moving=True)  # vs stationary preds
        X = np.stack([X0, X1])
        for h in range(2):
            sl = slice(h * HALF, (h + 1) * HALF)
            W0 = _aug_split(g[sl], gg[sl], moving=False)
            W1 = _aug_split(p[sl], pp[sl], moving=False)
            Wm = np.stack([W0, W1])
            in_maps.append(
                {"W": np.ascontiguousarray(Wm), "X": np.ascontiguousarray(X)}
            )
    return in_maps


def _make_in_maps_v3(gts, preds):
    in_maps = []
    for b in range(B):
        g = gts[b]  # [N, 3]
        p = preds[b]
        # moving side: orientation 0 scans preds, orientation 1 scans gts
        X = np.stack([p.T, g.T]).astype(np.float32)  # [2, 3, N]
        for h in range(2):
            sl = slice(h * HALF, (h + 1) * HALF)
            # stationary: [128, (s*3+d)*NRT + rt] = coord d of point rt*128+row
            gh = g[sl].reshape(NRT, PART, 3)  # [rt, p, d]
            ph = p[sl].reshape(NRT, PART, 3)
            Gm = np.empty((PART, 6 * NRT), dtype=np.float32)
            for d in range(3):
                Gm[:, (0 * 3 + d) * NRT : (0 * 3 + d) * NRT + NRT] = gh[:, :, d].T
                Gm[:, (1 * 3 + d) * NRT : (1 * 3 + d) * NRT + NRT] = ph[:, :, d].T
            in_maps.append(
                {"X": np.ascontiguousarray(X), "G": np.ascontiguousarray(Gm)}
            )
    return in_maps


def _make_in_maps(gts, preds, variant=None):
    variant = variant or VARIANT
    gts = np.asarray(gts, dtype=np.float32)
    preds = np.asarray(preds, dtype=np.float32)
    fn = {"v1": _make_in_maps_v1, "v2": _make_in_maps_v2, "v3": _make_in_maps_v3}[
        variant
    ]
    return fn(gts, preds)


def _combine(results):
    # OUT [128, 64]: col s*NRT + rt holds rowmins of stationary rows
    # rt*128 .. rt*128+127 for orientation s, on the core's half.
    loss = np.zeros(B, dtype=np.float32)
    for b in range(B):
        per_s = []
        for s in range(2):
            halves = []
            for h in range(2):
                out = results[2 * b + h]["OUT"]  # [128, 64]
                sub = out[:, s * NRT : (s + 1) * NRT]  # [128, 32]
                halves.append(sub.T.reshape(-1))  # [4096] rowmin d2
            per_s.append(np.concatenate(halves))  # [8192]
        d2 = np.concatenate(per_s)
        d = np.sqrt(np.maximum(d2.astype(np.float64), 0.0))
        loss[b] = np.float32(d.sum())
    return loss


def _get_cached_exec(variant, reps):
    """Build the PJRT executable for this bass module ONCE and reuse it.

    `run_bass_kernel_spmd` -> `run_bass_via_pjrt` creates a fresh
    `jax.jit(shard_map(...))` wrapper per call, so every call re-traces
    (~100ms). Replicating its multi-core branch with a cached jitted
    callable removes that per-call overhead.
    """
    key = ("exec", variant, reps)
    if key in _CACHE:
        return _CACHE[key]

    import jax
    import concourse.mybir as mybir
    from jax.sharding import Mesh, PartitionSpec
    from jax.experimental.shard_map import shard_map
    from concourse import bass2jax

    nc = get_nc(variant, reps)
    bass2jax.install_neuronx_cc_hook()
    n_cores = 8

    partition_name = nc.partition_id_tensor.name if nc.partition_id_tensor else None
    in_names, out_names, out_avals, zero_outs = [], [], [], []
    for alloc in nc.m.functions[0].allocations:
        if not isinstance(alloc, mybir.MemoryLocationSet):
            continue
        name = alloc.memorylocations[0].name
        if alloc.kind == "ExternalInput":
            if name != partition_name:
                in_names.append(name)
        elif alloc.kind == "ExternalOutput":
            out_names.append(name)
            shape = tuple(alloc.tensor_shape)
            dtype = mybir.dt.np(alloc.dtype)
            out_avals.append(jax.core.ShapedArray(shape, dtype))
            zero_outs.append(np.zeros(shape, dtype))
    n_params = len(in_names)
    all_names = in_names + out_names
    if partition_name is not None:
        all_names = all_names + [partition_name]

    def _body(*args):
        operands = list(args)
        if partition_name is not None:
            operands.append(bass2jax.partition_id_tensor())
        outs = bass2jax._bass_exec_p.bind(
            *operands,
            out_avals=tuple(out_avals),
            in_names=tuple(all_names),
            out_names=tuple(out_names),
            lowering_input_output_aliases=(),
            sim_require_finite=True,
            sim_require_nnan=True,
            nc=nc,
        )
        return tuple(outs)

    devices = jax.devices()[:n_cores]
    mesh = Mesh(np.asarray(devices), ("core",))
    donate = tuple(range(n_params, n_params + len(out_names)))
    sharded = jax.jit(
        shard_map(
            _body,
            mesh=mesh,
            in_specs=(PartitionSpec("core"),) * (n_params + len(out_names)),
            out_specs=(PartitionSpec("core"),) * len(out_names),
            check_rep=False,
        ),
        donate_argnums=donate,
        keep_unused=True,
    )
    entry = (sharded, in_names, out_names, out_avals, zero_outs, n_cores)
    _CACHE[key] = entry
    return entry


class _Res:
    def __init__(self, results):
        self.results = results


def _run(in_maps, variant=None, reps=1, trace=False):
    variant = variant or VARIANT
    sharded, in_names, out_names, out_avals, zero_outs, n_cores = _get_cached_exec(
        variant, reps
    )
    concat_in = [
        np.concatenate([np.asarray(m[name]) for m in in_maps], axis=0)
        for name in in_names
    ]
    concat_zeros = [
        np.zeros((n_cores * z.shape[0], *z.shape[1:]), z.dtype) for z in zero_outs
    ]
    out_arrs = sharded(*concat_in, *concat_zeros)
    results = [
        {
            name: np.asarray(out_arrs[i]).reshape(n_cores, *out_avals[i].shape)[c]
            for i, name in enumerate(out_names)
        }
        for c in range(n_cores)
    ]
    return _Res(results)


def kernel(gts, preds):
    in_maps = _make_in_maps(gts, preds)
    res = _run(in_maps)
    return _combine(res.results)
